# revision 5
# baseline (speedup 1.0000x reference)
"""MDCA loss kernel for Trainium2, 8 NeuronCores, data-parallel over batch.

reference:
    counts[c]   = histogram(target) ; avg_count = counts/B
    avg_conf    = mean(logits, axis=1)            # [E, C]
    loss[e]     = mean_c |avg_conf[e,c] - avg_count[c]|

Device computes ONLY per-core column sums of logits (the 16.4 MB/core
stream); the target histogram (8 KB) and the final abs/mean run on host.

Per core (batch shard of 1024 rows, partition p holds rows 8p..8p+7):
  - logits stream HBM->SBUF with an inline f32->bf16 DGE cast (SWDGE):
    HBM reads stay 2 MB/chunk but the SBUF-fabric side halves, lifting
    the stream above the 435 GB/s SBUF-AXI ceiling that binds a pure
    f32 HWDGE stream.  Chunks of 4 rows/partition = 16 KB contiguous
    HBM lines.  Optional hybrid plan mixes HWDGE f32 chunks.
  - ONE PSUM accumulation chain per 500-column half: bf16 selector
    matmuls ([128,4] with ones in column e) fold rows+partitions:
    psum[e,c] = sum over shard of logits[e,:,c]
  - tail: close bank0 first, copy on DVE + store on sync overlapping
    bank1's last matmuls, whose copy runs on ACT + store on scalar.
  - host: bincount(target), |sum_conf - counts|.mean / (B*C) -> loss[4]
"""

import os
import sys

for _p in ("/opt/trn_rl_repo", "/root/.axon_site/_ro/trn_rl_repo"):
    if os.path.isdir(_p) and _p not in sys.path:
        sys.path.insert(0, _p)

import numpy as np

import concourse.bass as bass
import concourse.bacc as bacc
import concourse.tile as tile
import concourse.mybir as mybir
from concourse.bass_utils import run_bass_kernel_spmd

E, B, C = 4, 8192, 1000
N_CORES = 8
BS = B // N_CORES          # 1024 batch rows per core
GP = 8                     # rows folded per partition (BS = 128 * GP)
CH = C // 2                # 500, C half per PSUM bank
F32 = mybir.dt.float32
BF16 = mybir.dt.bfloat16

# Load plan: list of (engine, exit, row0, row1, dtype) chunks, issued in
# order per engine.  engine: "g" = gpsimd SWDGE (casts when dtype=BF16),
# "s" = sync HWDGE, "c" = scalar HWDGE (f32 only).
# Default: pure SWDGE bf16, 4-row chunks, last two chunks 2-row for a
# short post-stream tail.
PLAN_SWDGE_BF16 = (
    [("g", e, r, r + 4, "bf16") for e in range(3) for r in (0, 4)]
    + [("g", 3, 0, 4, "bf16")]
    + [("g", 3, 4, 6, "bf16"), ("g", 3, 6, 8, "bf16")]
)

PLAN = PLAN_SWDGE_BF16


def build_nc(plan=None):
    plan = PLAN if plan is None else plan
    nc = bacc.Bacc(
        "TRN2",
        target_bir_lowering=False,
        debug=False,
        enable_asserts=False,
        num_devices=N_CORES,
    )

    logits = nc.dram_tensor("logits", [E, BS, C], F32, kind="ExternalInput")
    part_out = nc.dram_tensor("part", [E, C], F32, kind="ExternalOutput")

    # per-exit view: partition p holds rows 8p..8p+7
    src = [logits[e].rearrange("(p i) c -> p i c", i=GP) for e in range(E)]
    eng = {"g": nc.gpsimd, "s": nc.sync, "c": nc.scalar}

    with tile.TileContext(nc) as tc:
        with (
            tc.tile_pool(name="const", bufs=1) as const,
            tc.tile_pool(name="ld", bufs=len(plan)) as ld,
            tc.tile_pool(name="work", bufs=2) as work,
            tc.tile_pool(name="psum", bufs=1, space=bass.MemorySpace.PSUM) as psum,
        ):
            # ---- phase 1: every load DMA first so rings fill immediately
            tiles = []
            for k, (en, e, r0, r1, dt) in enumerate(plan):
                rows = r1 - r0
                dtype = BF16 if dt == "bf16" else F32
                t = ld.tile([128, rows * C], dtype, tag=f"ld{rows}_{dt}",
                            name=f"ld{k}_e{e}r{r0}")
                eng[en].dma_start(
                    out=t.rearrange("p (i c) -> p i c", i=rows),
                    in_=src[e][:, r0:r1, :],
                )
                tiles.append(t)

            # ---- phase 2: selector weights (bf16 and f32 variants)
            sels = const.tile([128, 4 * E], BF16, tag="sels")
            nc.vector.memset(sels[:], 0.0)
            for e in range(E):
                nc.vector.memset(sels[:, 4 * e + e : 4 * e + e + 1], 1.0)
            sels_f = const.tile([128, 4 * E], F32, tag="sels_f")
            nc.vector.tensor_copy(sels_f[:], sels[:])

            pbank = [
                psum.tile([E, CH], F32, tag=f"pc{h}", name=f"pc{h}")
                for h in range(2)
            ]

            # ---- phase 3: selector matmuls in arrival order.
            # Per chunk: all bank-0 matmuls then all bank-1 (so on the
            # final chunk bank0 closes early and its copy/store overlaps
            # bank1's tail matmuls).
            n_mm = [0, 0]
            tot_mm = sum(2 * (r1 - r0) for (_, _, r0, r1, _) in plan)
            for k, (en, e, r0, r1, dt) in enumerate(plan):
                rows = r1 - r0
                t = tiles[k]
                w = sels if dt == "bf16" else sels_f
                for h in range(2):
                    for i in range(rows):
                        nc.tensor.matmul(
                            pbank[h][:],
                            w[:, 4 * e : 4 * e + 4],
                            t[:, i * C + h * CH : i * C + (h + 1) * CH],
                            start=(n_mm[h] == 0),
                            stop=(n_mm[h] == tot_mm // 2 - 1),
                        )
                        n_mm[h] += 1

            # ---- phase 4: PSUM->SBUF on two engines + parallel stores
            sb0 = work.tile([E, CH], F32, tag="sb0")
            nc.vector.tensor_copy(sb0[:], pbank[0][:])
            nc.sync.dma_start(out=part_out[:, 0:CH], in_=sb0[:])
            sb1 = work.tile([E, CH], F32, tag="sb1")
            nc.scalar.copy(sb1[:], pbank[1][:])
            nc.scalar.dma_start(out=part_out[:, CH:C], in_=sb1[:])

    nc.compile()
    return nc


_NC_CACHE = {}


def _get_nc():
    if "nc" not in _NC_CACHE:
        _NC_CACHE["nc"] = build_nc()
    return _NC_CACHE["nc"]


def make_in_maps(logits: np.ndarray, target: np.ndarray):
    logits = np.ascontiguousarray(logits, dtype=np.float32)
    in_maps = []
    for c in range(N_CORES):
        lg = logits[:, c * BS : (c + 1) * BS, :]
        in_maps.append({"logits": np.ascontiguousarray(lg)})
    return in_maps


def kernel(logits: np.ndarray, target: np.ndarray) -> np.ndarray:
    nc = _get_nc()
    in_maps = make_in_maps(logits, target)
    res = run_bass_kernel_spmd(nc, in_maps, core_ids=list(range(N_CORES)))
    parts = sum(np.asarray(r["part"], dtype=np.float64) for r in res.results)
    counts = np.bincount(
        np.asarray(target).astype(np.int64), minlength=C
    ).astype(np.float64)
    return (np.abs(parts - counts[None, :]).sum(axis=1) / (B * C)).astype(
        np.float32
    )


# revision 7
# speedup vs baseline: 1.1284x; 1.1284x over previous
"""MDCA loss kernel for Trainium2, 8 NeuronCores, data-parallel over batch.

reference:
    counts[c]   = histogram(target) ; avg_count = counts/B
    avg_conf    = mean(logits, axis=1)            # [E, C]
    loss[e]     = mean_c |avg_conf[e,c] - avg_count[c]|

Device computes ONLY per-core column sums of logits (the 16.4 MB/core
stream, which binds at the ~430 GB/s per-core HBM-read ceiling); the
target histogram (8 KB) and final abs/mean run on host.  A bf16
DGE-cast variant was measured: the HBM-read side still caps ~420 GB/s
and SWDGE's Q7 descriptor emission adds a long trickle tail, so pure
HWDGE f32 wins.

Per core (batch shard of 1024 rows, partition p holds rows 8p..8p+7):
  - two HWDGE rings (sync 8.70 MB / scalar 7.68 MB), 16 KB contiguous
    lines for the 4-row chunks.  Chunk sizes taper [4,4,4,2,2,1] /
    [4,4,4,2,1] rows so the end-game is short: the byte imbalance makes
    scalar's ring drain first, its closer matmuls run before sync's
    final single-row closer lands, leaving ~3 us of post-stream work.
  - DVE folds row pairs into [128,1000] f32r tiles (1.2 us each, 15
    total, always ahead of arrivals); single-row closers skip DVE.
  - ONE PSUM chain per 500-column half: f32r selector matmuls
    ([128,4] with ones in column e) fold rows+partitions:
    psum[e,c] = sum over shard of logits[e,:,c]
  - tail: bank0 closes one matmul before bank1; copy0 on DVE + store
    on sync overlap bank1's close, whose copy runs on ACT + scalar.
  - host: bincount(target); |sum_conf - counts|.mean / (B*C) -> loss[4]
"""

import os
import sys

for _p in ("/opt/trn_rl_repo", "/root/.axon_site/_ro/trn_rl_repo"):
    if os.path.isdir(_p) and _p not in sys.path:
        sys.path.insert(0, _p)

import numpy as np

import concourse.bass as bass
import concourse.bacc as bacc
import concourse.tile as tile
import concourse.mybir as mybir
from concourse.bass_utils import run_bass_kernel_spmd

E, B, C = 4, 8192, 1000
N_CORES = 8
BS = B // N_CORES          # 1024 batch rows per core
GP = 8                     # rows folded per partition (BS = 128 * GP)
CH = C // 2                # 500, C half per PSUM bank
F32 = mybir.dt.float32
F32R = mybir.dt.float32r

# (ring, exit, row0, row1) in issue order per ring; interleaved arrival
# order below.  ring "s"=sync, "c"=scalar.
RING_S = [(0, 0, 4), (1, 0, 4), (2, 0, 4), (3, 0, 2), (3, 2, 4), (3, 4, 5)]
RING_C = [(0, 4, 8), (1, 4, 8), (2, 4, 8), (3, 6, 8), (3, 5, 6)]
# arrival order for compute: alternate rings, scalar's tail first
ORDER = ["s0", "c0", "s1", "c1", "s2", "c2", "s3", "c3", "c4", "s4", "s5"]


def build_nc():
    nc = bacc.Bacc(
        "TRN2",
        target_bir_lowering=False,
        debug=False,
        enable_asserts=False,
        num_devices=N_CORES,
    )

    logits = nc.dram_tensor("logits", [E, BS, C], F32, kind="ExternalInput")
    part_out = nc.dram_tensor("part", [E, C], F32, kind="ExternalOutput")

    # per-exit view: partition p holds rows 8p..8p+7
    src = [logits[e].rearrange("(p i) c -> p i c", i=GP) for e in range(E)]

    with tile.TileContext(nc) as tc:
        with (
            tc.tile_pool(name="const", bufs=1) as const,
            tc.tile_pool(name="ld4", bufs=6) as ld4,
            tc.tile_pool(name="ld2", bufs=3) as ld2,
            tc.tile_pool(name="ldz", bufs=2) as ldz,
            tc.tile_pool(name="fold", bufs=4) as foldp,
            tc.tile_pool(name="work", bufs=2) as work,
            tc.tile_pool(name="psum", bufs=1, space=bass.MemorySpace.PSUM) as psum,
        ):
            # ---- phase 1: every load DMA first so both rings fill and
            # stream back-to-back
            def ld_dma(engine, key, e, r0, r1):
                rows = r1 - r0
                if rows == 1:           # closer: f32r bitcast, no fold
                    pool, dt = ldz, F32R
                else:
                    pool, dt = (ld4 if rows == 4 else ld2), F32
                t = pool.tile([128, rows * C], dt, tag=f"ld{rows}",
                              name=f"ld_{key}")
                in_ = src[e][:, r0:r1, :]
                if dt is F32R:
                    in_ = in_.bitcast(F32R)
                engine.dma_start(
                    out=t.rearrange("p (i c) -> p i c", i=rows), in_=in_
                )
                return t

            tiles = {}
            for k, (e, r0, r1) in enumerate(RING_S):
                tiles[f"s{k}"] = (ld_dma(nc.sync, f"s{k}", e, r0, r1), e,
                                  r1 - r0)
            for k, (e, r0, r1) in enumerate(RING_C):
                tiles[f"c{k}"] = (ld_dma(nc.scalar, f"c{k}", e, r0, r1), e,
                                  r1 - r0)

            # ---- phase 2: selector weights
            sels_f = const.tile([128, 4 * E], F32, tag="sels_f")
            nc.vector.memset(sels_f[:], 0.0)
            for e in range(E):
                nc.vector.memset(sels_f[:, 4 * e + e : 4 * e + e + 1], 1.0)
            sels = const.tile([128, 4 * E], F32R, tag="sels")
            nc.vector.tensor_copy(sels[:], sels_f[:])

            pbank = [
                psum.tile([E, CH], F32, tag=f"pc{h}", name=f"pc{h}")
                for h in range(2)
            ]

            # ---- phase 3: folds + matmuls in expected arrival order
            n_mm = [0, 0]
            tot_mm = 2 * sum((r1 - r0) // 2 + (r1 - r0) % 2
                             for (_, r0, r1) in RING_S + RING_C)

            def mm(f, e):
                for h in range(2):
                    nc.tensor.matmul(
                        pbank[h][:],
                        sels[:, 4 * e : 4 * e + 4],
                        f[:, h * CH : (h + 1) * CH],
                        start=(n_mm[h] == 0),
                        stop=(n_mm[h] == tot_mm // 2 - 1),
                    )
                    n_mm[h] += 1

            for key in ORDER:
                t, e, rows = tiles[key]
                if rows == 1:
                    mm(t, e)
                    continue
                for g in range(rows // 2):
                    f = foldp.tile([128, C], F32R, tag="fold",
                                   name=f"f_{key}g{g}")
                    nc.vector.tensor_add(
                        f[:], t[:, 2 * g * C : (2 * g + 1) * C],
                        t[:, (2 * g + 1) * C : (2 * g + 2) * C],
                    )
                    mm(f, e)

            # ---- phase 4: PSUM->SBUF on two engines + parallel stores
            sb0 = work.tile([E, CH], F32, tag="sb0")
            nc.vector.tensor_copy(sb0[:], pbank[0][:])
            nc.sync.dma_start(out=part_out[:, 0:CH], in_=sb0[:])
            sb1 = work.tile([E, CH], F32, tag="sb1")
            nc.scalar.copy(sb1[:], pbank[1][:])
            nc.scalar.dma_start(out=part_out[:, CH:C], in_=sb1[:])

    nc.compile()
    return nc


_NC_CACHE = {}


def _get_nc():
    if "nc" not in _NC_CACHE:
        _NC_CACHE["nc"] = build_nc()
    return _NC_CACHE["nc"]


def make_in_maps(logits: np.ndarray, target: np.ndarray):
    logits = np.ascontiguousarray(logits, dtype=np.float32)
    in_maps = []
    for c in range(N_CORES):
        lg = logits[:, c * BS : (c + 1) * BS, :]
        in_maps.append({"logits": np.ascontiguousarray(lg)})
    return in_maps


def kernel(logits: np.ndarray, target: np.ndarray) -> np.ndarray:
    nc = _get_nc()
    in_maps = make_in_maps(logits, target)
    res = run_bass_kernel_spmd(nc, in_maps, core_ids=list(range(N_CORES)))
    parts = sum(np.asarray(r["part"], dtype=np.float64) for r in res.results)
    counts = np.bincount(
        np.asarray(target).astype(np.int64), minlength=C
    ).astype(np.float64)
    return (np.abs(parts - counts[None, :]).sum(axis=1) / (B * C)).astype(
        np.float32
    )


# revision 13
# speedup vs baseline: 1.1854x; 1.0505x over previous
"""MDCA loss kernel for Trainium2, 8 NeuronCores, data-parallel over batch.

reference:
    counts[c]   = histogram(target) ; avg_count = counts/B
    avg_conf    = mean(logits, axis=1)            # [E, C]
    loss[e]     = mean_c |avg_conf[e,c] - avg_count[c]|

Device computes ONLY per-core column sums of logits (the 16.4 MB/core
stream, which binds at the ~430 GB/s per-core HBM-read ceiling); the
target histogram (8 KB) and final abs/mean run on host.  A bf16
DGE-cast variant was measured: the HBM-read side still caps ~420 GB/s
and SWDGE's Q7 descriptor emission adds a long trickle tail, so pure
HWDGE f32 wins.

Per core (batch shard of 1024 rows, partition p holds rows 8p..8p+7):
  - two HWDGE rings (sync 8.70 MB / scalar 7.68 MB), 16 KB contiguous
    lines for the 4-row chunks.  Chunk sizes taper [4,4,4,2,2,1] /
    [4,4,4,2,1] rows so the end-game is short: the byte imbalance makes
    scalar's ring drain first, its closer matmuls run before sync's
    final single-row closer lands, leaving ~3 us of post-stream work.
  - DVE folds row pairs into [128,1000] f32r tiles (1.2 us each, 15
    total, always ahead of arrivals); single-row closers skip DVE.
  - ONE PSUM chain per 500-column half: f32r selector matmuls
    ([128,4] with ones in column e) fold rows+partitions:
    psum[e,c] = sum over shard of logits[e,:,c]
  - tail: bank0 closes one matmul before bank1; copy0 on DVE + store
    on sync overlap bank1's close, whose copy runs on ACT + scalar.
  - host: bincount(target); |sum_conf - counts|.mean / (B*C) -> loss[4]
"""

import os
import sys

for _p in ("/opt/trn_rl_repo", "/root/.axon_site/_ro/trn_rl_repo"):
    if os.path.isdir(_p) and _p not in sys.path:
        sys.path.insert(0, _p)

import numpy as np

import concourse.bass as bass
import concourse.bacc as bacc
import concourse.tile as tile
import concourse.mybir as mybir
from concourse.bass_utils import run_bass_kernel_spmd

E, B, C = 4, 8192, 1000
N_CORES = 8
BS = B // N_CORES          # 1024 batch rows per core
GP = 8                     # rows folded per partition (BS = 128 * GP)
CH = C // 2                # 500, C half per PSUM bank
F32 = mybir.dt.float32
F32R = mybir.dt.float32r
BF16 = mybir.dt.bfloat16

# (exit, row0, row1, col0, col1) in issue order per ring.  The scalar
# ring's HWDGE descriptor generation lags sync's by ~3 us when sync goes
# first, so scalar issues FIRST and carries the extra bytes.  Chunk rows
# taper [4,4,4,2,1,1,.5,.5] / [4,4,4,2,.5,.5] so the end-game is four
# half-row closers at one matmul each.
RING_C = [(0, 4, 8, 0, C), (1, 4, 8, 0, C), (2, 4, 8, 0, C),
          (3, 4, 6, 0, C), (3, 6, 7, 0, C), (3, 7, 8, 0, C),
          (3, 3, 4, 0, CH), (3, 3, 4, CH, C)]
RING_S = [(0, 0, 4, 0, C), (1, 0, 4, 0, C), (2, 0, 4, 0, C),
          (3, 0, 2, 0, C), (3, 2, 3, 0, CH), (3, 2, 3, CH, C)]
# compute order ~ arrival order (scalar starts ~3 us early; both rings
# drain at the same packet-fair rate; half-closers land last)
ORDER = ["c0", "s0", "c1", "s1", "c2", "s2", "c3", "s3", "c4", "c5",
         "s4", "c6", "s5", "c7"]


def build_nc():
    nc = bacc.Bacc(
        "TRN2",
        target_bir_lowering=False,
        debug=False,
        enable_asserts=False,
        num_devices=N_CORES,
    )

    logits = nc.dram_tensor("logits", [E, BS, C], F32, kind="ExternalInput")
    part_out = nc.dram_tensor("part", [E, C], F32, kind="ExternalOutput")

    # per-exit view: partition p holds rows 8p..8p+7
    src = [logits[e].rearrange("(p i) c -> p i c", i=GP) for e in range(E)]

    with tile.TileContext(nc) as tc:
        with (
            tc.tile_pool(name="const", bufs=1) as const,
            tc.tile_pool(name="ld4", bufs=6) as ld4,
            tc.tile_pool(name="ld2", bufs=3) as ld2,
            tc.tile_pool(name="ldz", bufs=4) as ldz,
            tc.tile_pool(name="fold", bufs=4) as foldp,
            tc.tile_pool(name="work", bufs=2) as work,
            tc.tile_pool(name="psum", bufs=1, space=bass.MemorySpace.PSUM) as psum,
        ):
            # ---- phase 1: every load DMA first so both rings fill and
            # stream back-to-back
            def ld_dma(engine, key, e, r0, r1, c0, c1):
                rows, cols = r1 - r0, c1 - c0
                if rows == 1:           # closer: f32r bitcast, no fold
                    pool, dt = ldz, F32R
                else:
                    pool, dt = (ld4 if rows == 4 else ld2), F32
                t = pool.tile([128, rows * cols], dt,
                              tag=f"ld{rows}_{cols}", name=f"ld_{key}")
                in_ = src[e][:, r0:r1, c0:c1]
                if dt is F32R:
                    in_ = in_.bitcast(F32R)
                engine.dma_start(
                    out=t.rearrange("p (i c) -> p i c", i=rows), in_=in_
                )
                return t

            tiles = {}
            for k, spec in enumerate(RING_C):
                tiles[f"c{k}"] = (ld_dma(nc.scalar, f"c{k}", *spec), spec)
            for k, spec in enumerate(RING_S):
                tiles[f"s{k}"] = (ld_dma(nc.sync, f"s{k}", *spec), spec)

            # ---- phase 2: selector weights (bf16 for folded tiles,
            # f32r for closers)
            sels_b = const.tile([128, 4 * E], BF16, tag="sels_b")
            nc.vector.memset(sels_b[:], 0.0)
            for e in range(E):
                nc.vector.memset(sels_b[:, 4 * e + e : 4 * e + e + 1], 1.0)
            sels_f = const.tile([128, 4 * E], F32, tag="sels_f")
            nc.vector.tensor_copy(sels_f[:], sels_b[:])
            sels_r_t = const.tile([128, 4 * E], F32R, tag="sels_r")
            nc.vector.tensor_copy(sels_r_t[:], sels_f[:])
            sels_r = sels_r_t[:]

            pbank = [
                psum.tile([E, CH], F32, tag=f"pc{h}", name=f"pc{h}")
                for h in range(2)
            ]

            # ---- phase 3: folds + matmuls in expected arrival order.
            # Precount per-bank matmuls so the last one sets stop=True.
            tot = [0, 0]
            for _, r0, r1, c0, c1 in RING_S + RING_C:
                rows = r1 - r0
                if c1 - c0 == CH:
                    tot[c0 // CH] += 1
                else:
                    tot[0] += rows // 2 + rows % 2
                    tot[1] += rows // 2 + rows % 2
            n_mm = [0, 0]

            def mm1(h, w, data):
                nc.tensor.matmul(
                    pbank[h][:], w, data,
                    start=(n_mm[h] == 0),
                    stop=(n_mm[h] == tot[h] - 1),
                )
                n_mm[h] += 1

            for key in ORDER:
                t, (e, r0, r1, c0, c1) = tiles[key]
                rows = r1 - r0
                wb = sels_b[:, 4 * e : 4 * e + 4]
                wr = sels_r[:, 4 * e : 4 * e + 4]
                if rows == 1 and c1 - c0 == CH:     # half-row closer
                    mm1(c0 // CH, wr, t[:, 0:CH])
                elif rows == 1:                     # full-row closer
                    for h in range(2):
                        mm1(h, wr, t[:, h * CH : (h + 1) * CH])
                else:
                    for g in range(rows // 2):
                        f = foldp.tile([128, C], BF16, tag="fold",
                                       name=f"f_{key}g{g}")
                        nc.vector.tensor_add(
                            f[:], t[:, 2 * g * C : (2 * g + 1) * C],
                            t[:, (2 * g + 1) * C : (2 * g + 2) * C],
                        )
                        for h in range(2):
                            mm1(h, wb, f[:, h * CH : (h + 1) * CH])

            # ---- phase 4: PSUM->SBUF on two engines + parallel stores
            sb0 = work.tile([E, CH], F32, tag="sb0")
            nc.vector.tensor_copy(sb0[:], pbank[0][:])
            nc.sync.dma_start(out=part_out[:, 0:CH], in_=sb0[:])
            sb1 = work.tile([E, CH], F32, tag="sb1")
            nc.scalar.copy(sb1[:], pbank[1][:])
            nc.scalar.dma_start(out=part_out[:, CH:C], in_=sb1[:])

    nc.compile()
    return nc


_NC_CACHE = {}


def _get_nc():
    if "nc" not in _NC_CACHE:
        _NC_CACHE["nc"] = build_nc()
    return _NC_CACHE["nc"]


def make_in_maps(logits: np.ndarray, target: np.ndarray):
    logits = np.ascontiguousarray(logits, dtype=np.float32)
    in_maps = []
    for c in range(N_CORES):
        lg = logits[:, c * BS : (c + 1) * BS, :]
        in_maps.append({"logits": np.ascontiguousarray(lg)})
    return in_maps


def kernel(logits: np.ndarray, target: np.ndarray) -> np.ndarray:
    nc = _get_nc()
    in_maps = make_in_maps(logits, target)
    res = run_bass_kernel_spmd(nc, in_maps, core_ids=list(range(N_CORES)))
    parts = sum(np.asarray(r["part"], dtype=np.float64) for r in res.results)
    counts = np.bincount(
        np.asarray(target).astype(np.int64), minlength=C
    ).astype(np.float64)
    return (np.abs(parts - counts[None, :]).sum(axis=1) / (B * C)).astype(
        np.float32
    )
